# revision 13
# baseline (speedup 1.0000x reference)
"""Trainium2 Bass kernel for nn_DiffModule_40827959116531 (sparse_attention).

Reference (per batch element):
    sv  = src @ W1 + b1;  tk = trg @ W2 + b2;  tv = trg @ W1 + b1
    score = sv @ tk.T / sqrt(O);  prob = softmax(score)
    ctx = prob @ tv;  h = [sv, sv - ctx]
    out = relu(h @ W3a + b3a) @ W3b + b3b

Algebraic restructuring (host-precomputed fused weights; exact up to the
shift-invariance of softmax which absorbs the b2 term):
    W12   = W1 @ W2.T               score = (src @ W12) @ trg.T + beta
    beta  = trg @ (W2 @ b1) + b1.b2         (per-target logit bias)
    Wfuse = W1 @ (W3aTop + W3aBot)  h @ W3a = src@Wfuse - ctx@W3aBot + bias
    Wcorr = W1 @ W3aBot             ctx@W3aBot = ((e@trg)/denom) @ Wcorr + ..
    bh2   = b1 @ W3aTop + b3a
This cuts the 8 matmul-units/batch to 6, and the 4 units feeding only the
softmax/correction path (g, score, ctxd, corr) tolerate fp8 -> run them as
fp8e4 DoubleRow (2 K-chunks per instruction). Only pre=src@Wfuse and the
final h2@W3b stay bf16. Verified vs reference in numpy: rel ~4e-3.

Scaling (fp8e4 min-normal 2^-6, TRN max +-240): W12 pre-scaled 4096, Wcorr
256; g stored x8 (drain scale 2^-9); exp drain scale 2^-8; ctxd stored
x0.5. The denominator ones-matrix holds 128.0 so reciprocal(d_ps) is
exactly the corr multiplier 1/(128*denom) broadcast on all partitions.

Sharding: data-parallel over B=32 across 8 cores (4 batch elems each).
"""

import math
from contextlib import ExitStack

import ml_dtypes
import numpy as np

import concourse.bass as bass
import concourse.mybir as mybir
import concourse.tile as tile
from concourse import bacc
from concourse.bass_utils import run_bass_kernel_spmd

P = 128
B_FULL = 32
N_CORES = 8
BS = B_FULL // N_CORES  # 4 batch elements per core
L = 1024
N = 1024
D = 1024
O = 1024

F32 = mybir.dt.float32
BF16 = mybir.dt.bfloat16
FP8 = mybir.dt.float8e4
AF = mybir.ActivationFunctionType
DR = mybir.MatmulPerfMode.DoubleRow
NP_BF16 = ml_dtypes.bfloat16
NP_FP8 = ml_dtypes.float8_e4m3fn

LC = 512
N_LC = L // LC            # 2 moving chunks of 512
KT = 8                    # 128-tiles along any contraction dim
KP = KT // 2              # DoubleRow pairs

WS = 4096.0               # host pre-scale on W12
WCS = 256.0               # host pre-scale on Wcorr
GS = 8.0                  # g storage scale
CS = 0.5                  # ctxd storage scale
G_DRAIN = GS / WS                     # 2^-9
E_DRAIN = 1.0 / (GS * math.sqrt(O))   # 2^-8


def _load_w(nc, dst, w_dram, ktiles):
    for k in range(ktiles):
        nc.sync.dma_start(dst[:, k, :], w_dram.ap()[k * P:(k + 1) * P, :])


def _load_act(nc, dest, dram, b):
    for k in range(KT):
        nc.sync.dma_start(dest[:, k, :], dram.ap()[b, k * P:(k + 1) * P, :])


def _build(nc, tc):
    src8_d = nc.dram_tensor("srcT8", [BS, D, L], FP8, kind="ExternalInput")
    srcb_d = nc.dram_tensor("srcTb", [BS, D, L], BF16, kind="ExternalInput")
    trgT_d = nc.dram_tensor("trgT8", [BS, D, N], FP8, kind="ExternalInput")
    trgN_d = nc.dram_tensor("trgN8", [BS, N, D], FP8, kind="ExternalInput")
    w12_d = nc.dram_tensor("W12s", [KT, D, P], FP8, kind="ExternalInput")
    wfuse_d = nc.dram_tensor("Wfuse", [D, O], BF16, kind="ExternalInput")
    wcorr_d = nc.dram_tensor("Wcorrs", [D, O], FP8, kind="ExternalInput")
    w3b_d = nc.dram_tensor("W3bb", [O, O], BF16, kind="ExternalInput")
    bh2_d = nc.dram_tensor("bh2", [O], F32, kind="ExternalInput")
    b3b_d = nc.dram_tensor("b3bf", [O], F32, kind="ExternalInput")
    beta_d = nc.dram_tensor("beta", [BS, N], F32, kind="ExternalInput")
    out = nc.dram_tensor("out", [BS, L, O], F32, kind="ExternalOutput")

    ctx = ExitStack()
    singles = ctx.enter_context(tc.tile_pool(name="singles", bufs=1))
    stp8 = ctx.enter_context(tc.tile_pool(name="stp8", bufs=2))
    stp1 = ctx.enter_context(tc.tile_pool(name="stp1", bufs=1))
    actp = ctx.enter_context(tc.tile_pool(name="actp", bufs=1))
    smallp = ctx.enter_context(tc.tile_pool(name="smallp", bufs=2))
    outp = ctx.enter_context(tc.tile_pool(name="outp", bufs=6))
    psum = ctx.enter_context(tc.tile_pool(name="psum", bufs=3, space="PSUM"))
    auxps = ctx.enter_context(tc.tile_pool(name="auxps", bufs=2, space="PSUM"))

    # ---- constants ----
    w12 = singles.tile([P, KT, D], FP8)
    wfuse = singles.tile([P, KT, O], BF16)
    wcorr = singles.tile([P, KT, O], FP8)
    w3b = singles.tile([P, KT, O], BF16)
    bh2col = singles.tile([P, KT], F32)
    b3bfull = singles.tile([P, O], F32)
    betafull = singles.tile([P, BS * KT], F32)
    onesbig = singles.tile([P, 2, N], FP8)

    nc.sync.dma_start(bh2col[:], bh2_d.ap().rearrange("(oo op) -> op oo", op=P))
    nc.sync.dma_start(
        betafull[:], beta_d.ap().rearrange("b (no np) -> np (b no)", np=P))
    nc.sync.dma_start(
        b3bfull[:], bass.AP(tensor=b3b_d.ap().tensor, offset=0, ap=[[0, P], [1, O]]))
    nc.vector.memset(onesbig[:], 128.0)

    def phase_a(s8, g8, lcs):
        """g_T[d2, l] = W12s.T @ srcT8, drain x 2^-9 -> fp8 (x GS)."""
        for j in range(KT):
            pss = {lcx: psum.tile([P, LC], F32, name=f"ps{lcx}", tag=f"ps{lcx}")
                   for lcx in lcs}
            for kp in range(KP):
                for lc in lcs:
                    nc.tensor.matmul(
                        pss[lc][:], w12[:, 2 * kp:2 * kp + 2, j * P:(j + 1) * P],
                        s8[:, 2 * kp:2 * kp + 2, lc * LC:(lc + 1) * LC],
                        start=(kp == 0), stop=(kp == KP - 1), perf_mode=DR)
            for lc in lcs:
                nc.scalar.activation(
                    g8[:, j, lc * LC:(lc + 1) * LC], pss[lc][:], AF.Identity,
                    scale=G_DRAIN)

    # batch-0: W12 block 0 + the first half of srcT8 go first; subsequent
    # W12 column-blocks are emitted between A(j) groups so A(j) only waits
    # for blocks <= j. Junk matmuls on the resident ones tile warm the HAM
    # clock gate during the initial DMA wait.
    s8_0 = stp8.tile([P, KT, L], FP8, tag="s8")
    t8_0 = stp8.tile([P, KT, N], FP8, tag="t8")
    nc.sync.dma_start(
        w12[:, :, 0:P], w12_d.ap()[0].rearrange("(kk p) c -> p kk c", p=P))
    for k in range(KT):
        nc.sync.dma_start(
            s8_0[:, k, 0:LC], src8_d.ap()[0, k * P:(k + 1) * P, 0:LC])
    warm_ps = psum.tile([P, LC], F32, name="ps0", tag="ps0")
    for _ in range(10):
        nc.tensor.matmul(warm_ps[:, :256], onesbig[:, :, :P],
                         onesbig[:, :, :256],
                         start=True, stop=True, perf_mode=DR)
    g8_0 = actp.tile([P, KT, L], FP8, tag="g8")
    for j in range(KT):
        if j + 1 < KT:
            nc.sync.dma_start(
                w12[:, :, (j + 1) * P:(j + 2) * P],
                w12_d.ap()[j + 1].rearrange("(kk p) c -> p kk c", p=P))
        ps = psum.tile([P, LC], F32, name="ps0", tag="ps0")
        for kp in range(KP):
            nc.tensor.matmul(
                ps[:], w12[:, 2 * kp:2 * kp + 2, j * P:(j + 1) * P],
                s8_0[:, 2 * kp:2 * kp + 2, 0:LC],
                start=(kp == 0), stop=(kp == KP - 1), perf_mode=DR)
        nc.scalar.activation(g8_0[:, j, 0:LC], ps[:], AF.Identity,
                             scale=G_DRAIN)
    for k in range(KT):
        nc.sync.dma_start(
            s8_0[:, k, LC:L], src8_d.ap()[0, k * P:(k + 1) * P, LC:L])
    phase_a(s8_0, g8_0, [1])
    _load_act(nc, t8_0, trgT_d, 0)
    tn_0 = stp1.tile([P, KT, N], FP8, tag="tn", bufs=2)
    _load_act(nc, tn_0, trgN_d, 0)
    _load_w(nc, wfuse, wfuse_d, KT)
    sb_0 = stp1.tile([P, KT, L], BF16, tag="sb")
    _load_act(nc, sb_0, srcb_d, 0)
    _load_w(nc, wcorr, wcorr_d, KT)
    _load_w(nc, w3b, w3b_d, KT)

    nxt = dict(s8=s8_0, t8=t8_0, sb=sb_0, tn=tn_0)
    for b in range(BS):
        s8, t8, sb, tn = nxt["s8"], nxt["t8"], nxt["sb"], nxt["tn"]

        # ---- A ----
        if b == 0:
            g8 = g8_0   # emitted above, interleaved with the prologue DMAs
        else:
            g8 = actp.tile([P, KT, L], FP8, tag="g8")
            phase_a(s8, g8, list(range(N_LC)))

        # ---- B: score_T[n, l]; e = exp(score/32 + beta) -> fp8 ----
        e8 = actp.tile([P, KT, L], FP8, tag="e8")
        for i in range(KT):
            pss = [psum.tile([P, LC], F32, name=f"ps{lcx}", tag=f"ps{lcx}")
                   for lcx in range(N_LC)]
            for kp in range(KP):
                for lc in range(N_LC):
                    nc.tensor.matmul(
                        pss[lc][:], t8[:, 2 * kp:2 * kp + 2, i * P:(i + 1) * P],
                        g8[:, 2 * kp:2 * kp + 2, lc * LC:(lc + 1) * LC],
                        start=(kp == 0), stop=(kp == KP - 1), perf_mode=DR)
            for lc in range(N_LC):
                nc.scalar.activation(
                    e8[:, i, lc * LC:(lc + 1) * LC], pss[lc][:], AF.Exp,
                    scale=E_DRAIN, bias=betafull[:, b * KT + i:b * KT + i + 1])

        # ---- C: ctxd_T[d, l] = trgN8.T @ e8, drain x 0.5 -> fp8 ----
        cx8 = actp.tile([P, KT, L], FP8, tag="cx8")
        for j in range(KT):
            pss = [psum.tile([P, LC], F32, name=f"ps{lcx}", tag=f"ps{lcx}")
                   for lcx in range(N_LC)]
            for ip in range(KP):
                for lc in range(N_LC):
                    nc.tensor.matmul(
                        pss[lc][:], tn[:, 2 * ip:2 * ip + 2, j * P:(j + 1) * P],
                        e8[:, 2 * ip:2 * ip + 2, lc * LC:(lc + 1) * LC],
                        start=(ip == 0), stop=(ip == KP - 1), perf_mode=DR)
            for lc in range(N_LC):
                nc.scalar.activation(
                    cx8[:, j, lc * LC:(lc + 1) * LC], pss[lc][:], AF.Identity,
                    scale=CS)

        if b + 1 < BS:
            nxt = dict(
                s8=stp8.tile([P, KT, L], FP8, name="s8n", tag="s8"),
                t8=stp8.tile([P, KT, N], FP8, name="t8n", tag="t8"),
                tn=stp1.tile([P, KT, N], FP8, name="tnn", tag="tn", bufs=2),
                sb=stp1.tile([P, KT, L], BF16, name="sbn", tag="sb"))
            _load_act(nc, nxt["s8"], src8_d, b + 1)
            _load_act(nc, nxt["t8"], trgT_d, b + 1)
            _load_act(nc, nxt["tn"], trgN_d, b + 1)
            _load_act(nc, nxt["sb"], srcb_d, b + 1)

        # ---- E: pre_T[o, l] = Wfuse.T @ srcTb + bh2 (bf16, independent) ----
        pre = actp.tile([P, KT, L], BF16, tag="pre")
        for j in range(KT):
            pss = [psum.tile([P, LC], F32, name=f"ps{lcx}", tag=f"ps{lcx}")
                   for lcx in range(N_LC)]
            for k in range(KT):
                for lc in range(N_LC):
                    nc.tensor.matmul(
                        pss[lc][:], wfuse[:, k, j * P:(j + 1) * P],
                        sb[:, k, lc * LC:(lc + 1) * LC],
                        start=(k == 0), stop=(k == KT - 1))
            for lc in range(N_LC):
                nc.scalar.activation(
                    pre[:, j, lc * LC:(lc + 1) * LC], pss[lc][:], AF.Identity,
                    bias=bh2col[:, j:j + 1])

        # denominator: DR ones-matrix partition-reduce (2 N-tiles per MM);
        # every d_ps row holds 128*denom, so the reciprocal lands already
        # broadcast: rbc = 2^-7/denom (2^-7 folds the Wcorr/ctxd scales).
        rbcs = []
        for lc in range(N_LC):
            d_ps = auxps.tile([P, LC], F32, tag="dps")
            for ip in range(KP):
                nc.tensor.matmul(
                    d_ps[:], onesbig[:, :, :P],
                    e8[:, 2 * ip:2 * ip + 2, lc * LC:(lc + 1) * LC],
                    start=(ip == 0), stop=(ip == KP - 1), perf_mode=DR)
            rbc = smallp.tile([P, LC], F32, tag="rbc")
            nc.vector.reciprocal(rbc[:], d_ps[:])
            rbcs.append(rbc)

        # ---- D: corr; h2 = relu(pre - corr/denom) -> bf16 ----
        h2 = actp.tile([P, KT, L], BF16, tag="h2")
        for j in range(KT):
            pss = [psum.tile([P, LC], F32, name=f"ps{lcx}", tag=f"ps{lcx}")
                   for lcx in range(N_LC)]
            for kp in range(KP):
                for lc in range(N_LC):
                    nc.tensor.matmul(
                        pss[lc][:], wcorr[:, 2 * kp:2 * kp + 2, j * P:(j + 1) * P],
                        cx8[:, 2 * kp:2 * kp + 2, lc * LC:(lc + 1) * LC],
                        start=(kp == 0), stop=(kp == KP - 1), perf_mode=DR)
            for lc in range(N_LC):
                lsl = slice(lc * LC, (lc + 1) * LC)
                tmp = smallp.tile([P, LC], F32, tag="tmp")
                nc.vector.tensor_mul(tmp[:], pss[lc][:], rbcs[lc][:])
                hsum = smallp.tile([P, LC], F32, tag="hsum")
                nc.vector.tensor_sub(hsum[:], pre[:, j, lsl], tmp[:])
                nc.scalar.activation(h2[:, j, lsl], hsum[:], AF.Relu)

        # ---- F: out[l, o] = h2.T @ W3b + b3b ----
        for lc in range(N_LC):
            for lt in range(LC // P):
                lab = lc * LC + lt * P
                pss = [psum.tile([P, LC], F32, name=f"ps{lcx}", tag=f"ps{lcx}")
                       for lcx in range(N_LC)]
                for k in range(KT):
                    for oc in range(O // LC):
                        nc.tensor.matmul(
                            pss[oc][:], h2[:, k, lab:lab + P],
                            w3b[:, k, oc * LC:(oc + 1) * LC],
                            start=(k == 0), stop=(k == KT - 1))
                for oc in range(O // LC):
                    o_sb = outp.tile([P, LC], F32, tag="osb")
                    nc.vector.tensor_add(o_sb[:], pss[oc][:],
                                         b3bfull[:, oc * LC:(oc + 1) * LC])
                    nc.sync.dma_start(
                        out.ap()[b, lab:lab + P, oc * LC:(oc + 1) * LC], o_sb[:])

    ctx.close()


_NC_CACHE = None


def _get_nc():
    global _NC_CACHE
    if _NC_CACHE is None:
        nc = bacc.Bacc("TRN2", target_bir_lowering=False, debug=False,
                       num_devices=N_CORES)
        with tile.TileContext(nc) as tc:
            _build(nc, tc)
        nc.compile()
        _NC_CACHE = nc
    return _NC_CACHE


def _q8(x, scale=1.0):
    y = np.asarray(x, np.float32) * np.float32(scale)
    np.clip(y, -240.0, 240.0, out=y)
    return y.astype(NP_FP8)


def kernel(**inputs):
    nc = _get_nc()
    src = np.asarray(inputs["src"], dtype=np.float32)
    trg = np.asarray(inputs["trg"], dtype=np.float32)
    W1 = np.asarray(inputs["W1"], np.float32)
    b1 = np.asarray(inputs["b1"], np.float32)
    W2 = np.asarray(inputs["W2"], np.float32)
    b2 = np.asarray(inputs["b2"], np.float32)
    W3a = np.asarray(inputs["W3a"], np.float32)
    b3a = np.asarray(inputs["b3a"], np.float32)
    W3b = np.asarray(inputs["W3b"], np.float32)
    b3b = np.asarray(inputs["b3b"], np.float32)

    W3aT, W3aB = W3a[:O], W3a[O:]
    W12 = W1 @ W2.T
    Wfuse = W1 @ (W3aT + W3aB)
    Wcorr = W1 @ W3aB
    bh2 = b1 @ W3aT + b3a
    beta = (trg @ (W2 @ b1) + np.dot(b1, b2)).astype(np.float32)  # (B, N)

    src_t = np.ascontiguousarray(src.transpose(0, 2, 1))   # (B, D, L)
    trg_t = np.ascontiguousarray(trg.transpose(0, 2, 1))   # (B, D, N)
    shared = {
        "W12s": np.ascontiguousarray(
            _q8(W12, WS).reshape(D, KT, P).transpose(1, 0, 2)),
        "Wfuse": np.ascontiguousarray(Wfuse.astype(NP_BF16)),
        "Wcorrs": np.ascontiguousarray(_q8(Wcorr, WCS)),
        "W3bb": np.ascontiguousarray(W3b.astype(NP_BF16)),
        "bh2": np.ascontiguousarray(bh2),
        "b3bf": np.ascontiguousarray(b3b),
    }
    src_t8 = _q8(src_t)
    src_tb = src_t.astype(NP_BF16)
    trg_t8 = _q8(trg_t)
    trg_n8 = _q8(trg)
    in_maps = []
    for c in range(N_CORES):
        m = dict(shared)
        s = slice(c * BS, (c + 1) * BS)
        m["srcT8"] = src_t8[s]
        m["srcTb"] = src_tb[s]
        m["trgT8"] = trg_t8[s]
        m["trgN8"] = trg_n8[s]
        m["beta"] = np.ascontiguousarray(beta[s])
        in_maps.append(m)
    res = run_bass_kernel_spmd(nc, in_maps, core_ids=list(range(N_CORES)))
    return np.concatenate([r["out"] for r in res.results], axis=0)


# revision 14
# speedup vs baseline: 1.0043x; 1.0043x over previous
"""Trainium2 Bass kernel for nn_DiffModule_40827959116531 (sparse_attention).

Reference (per batch element):
    sv  = src @ W1 + b1;  tk = trg @ W2 + b2;  tv = trg @ W1 + b1
    score = sv @ tk.T / sqrt(O);  prob = softmax(score)
    ctx = prob @ tv;  h = [sv, sv - ctx]
    out = relu(h @ W3a + b3a) @ W3b + b3b

Algebraic restructuring (host-precomputed fused weights; exact up to the
shift-invariance of softmax which absorbs the b2 term):
    W12   = W1 @ W2.T               score = (src @ W12) @ trg.T + beta
    beta  = trg @ (W2 @ b1) + b1.b2         (per-target logit bias)
    Wfuse = W1 @ (W3aTop + W3aBot)  h @ W3a = src@Wfuse - ctx@W3aBot + bias
    Wcorr = W1 @ W3aBot             ctx@W3aBot = ((e@trg)/denom) @ Wcorr + ..
    bh2   = b1 @ W3aTop + b3a
This cuts the 8 matmul-units/batch to 6, and the 4 units feeding only the
softmax/correction path (g, score, ctxd, corr) tolerate fp8 -> run them as
fp8e4 DoubleRow (2 K-chunks per instruction). Only pre=src@Wfuse and the
final h2@W3b stay bf16. Verified vs reference in numpy: rel ~4e-3.

Scaling (fp8e4 min-normal 2^-6, TRN max +-240): W12 pre-scaled 4096, Wcorr
256; g stored x8 (drain scale 2^-9); exp drain scale 2^-8; ctxd stored
x0.5. The denominator ones-matrix holds 128.0 so reciprocal(d_ps) is
exactly the corr multiplier 1/(128*denom) broadcast on all partitions.

Sharding: data-parallel over B=32 across 8 cores (4 batch elems each).
"""

import math
from contextlib import ExitStack

import ml_dtypes
import numpy as np

import concourse.bass as bass
import concourse.mybir as mybir
import concourse.tile as tile
from concourse import bacc
from concourse.bass_utils import run_bass_kernel_spmd

P = 128
B_FULL = 32
N_CORES = 8
BS = B_FULL // N_CORES  # 4 batch elements per core
L = 1024
N = 1024
D = 1024
O = 1024

F32 = mybir.dt.float32
BF16 = mybir.dt.bfloat16
FP8 = mybir.dt.float8e4
AF = mybir.ActivationFunctionType
DR = mybir.MatmulPerfMode.DoubleRow
NP_BF16 = ml_dtypes.bfloat16
NP_FP8 = ml_dtypes.float8_e4m3fn

LC = 512
N_LC = L // LC            # 2 moving chunks of 512
KT = 8                    # 128-tiles along any contraction dim
KP = KT // 2              # DoubleRow pairs

WS = 4096.0               # host pre-scale on W12
WCS = 256.0               # host pre-scale on Wcorr
GS = 8.0                  # g storage scale
CS = 0.5                  # ctxd storage scale
G_DRAIN = GS / WS                     # 2^-9
E_DRAIN = 1.0 / (GS * math.sqrt(O))   # 2^-8


def _load_w(nc, dst, w_dram, ktiles):
    for k in range(ktiles):
        nc.sync.dma_start(dst[:, k, :], w_dram.ap()[k * P:(k + 1) * P, :])


def _load_act(nc, dest, dram, b):
    for k in range(KT):
        nc.sync.dma_start(dest[:, k, :], dram.ap()[b, k * P:(k + 1) * P, :])


def _build(nc, tc):
    src8_d = nc.dram_tensor("srcT8", [BS, D, L], FP8, kind="ExternalInput")
    srcb_d = nc.dram_tensor("srcTb", [BS, D, L], BF16, kind="ExternalInput")
    trgT_d = nc.dram_tensor("trgT8", [BS, D, N], FP8, kind="ExternalInput")
    trgN_d = nc.dram_tensor("trgN8", [BS, N, D], FP8, kind="ExternalInput")
    w12_d = nc.dram_tensor("W12s", [KT, D, P], FP8, kind="ExternalInput")
    wfuse_d = nc.dram_tensor("Wfuse", [D, O], BF16, kind="ExternalInput")
    wcorr_d = nc.dram_tensor("Wcorrs", [D, O], FP8, kind="ExternalInput")
    w3b_d = nc.dram_tensor("W3bb", [O, O], BF16, kind="ExternalInput")
    bh2_d = nc.dram_tensor("bh2", [O], F32, kind="ExternalInput")
    b3b_d = nc.dram_tensor("b3bf", [O], F32, kind="ExternalInput")
    beta_d = nc.dram_tensor("beta", [BS, N], F32, kind="ExternalInput")
    out = nc.dram_tensor("out", [BS, L, O], F32, kind="ExternalOutput")

    ctx = ExitStack()
    singles = ctx.enter_context(tc.tile_pool(name="singles", bufs=1))
    stp8 = ctx.enter_context(tc.tile_pool(name="stp8", bufs=2))
    stp1 = ctx.enter_context(tc.tile_pool(name="stp1", bufs=1))
    actp = ctx.enter_context(tc.tile_pool(name="actp", bufs=1))
    smallp = ctx.enter_context(tc.tile_pool(name="smallp", bufs=2))
    outp = ctx.enter_context(tc.tile_pool(name="outp", bufs=6))
    psum = ctx.enter_context(tc.tile_pool(name="psum", bufs=3, space="PSUM"))
    auxps = ctx.enter_context(tc.tile_pool(name="auxps", bufs=2, space="PSUM"))

    # ---- constants ----
    w12 = singles.tile([P, KT, D], FP8)
    wfuse = singles.tile([P, KT, O], BF16)
    wcorr = singles.tile([P, KT, O], FP8)
    w3b = singles.tile([P, KT, O], BF16)
    bh2col = singles.tile([P, KT], F32)
    b3bfull = singles.tile([P, O], F32)
    betafull = singles.tile([P, BS * KT], F32)
    onesbig = singles.tile([P, 2, N], FP8)

    nc.sync.dma_start(bh2col[:], bh2_d.ap().rearrange("(oo op) -> op oo", op=P))
    nc.sync.dma_start(
        betafull[:], beta_d.ap().rearrange("b (no np) -> np (b no)", np=P))
    nc.sync.dma_start(
        b3bfull[:], bass.AP(tensor=b3b_d.ap().tensor, offset=0, ap=[[0, P], [1, O]]))
    nc.vector.memset(onesbig[:], 128.0)

    def phase_a(s8, g8, lcs):
        """g_T[d2, l] = W12s.T @ srcT8, drain x 2^-9 -> fp8 (x GS)."""
        for j in range(KT):
            pss = {lcx: psum.tile([P, LC], F32, name=f"ps{lcx}", tag=f"ps{lcx}")
                   for lcx in lcs}
            for kp in range(KP):
                for lc in lcs:
                    nc.tensor.matmul(
                        pss[lc][:], w12[:, 2 * kp:2 * kp + 2, j * P:(j + 1) * P],
                        s8[:, 2 * kp:2 * kp + 2, lc * LC:(lc + 1) * LC],
                        start=(kp == 0), stop=(kp == KP - 1), perf_mode=DR)
            for lc in lcs:
                nc.scalar.activation(
                    g8[:, j, lc * LC:(lc + 1) * LC], pss[lc][:], AF.Identity,
                    scale=G_DRAIN)

    # batch-0: W12 block 0 + the first half of srcT8 go first; subsequent
    # W12 column-blocks are emitted between A(j) groups so A(j) only waits
    # for blocks <= j. Junk matmuls on the resident ones tile warm the HAM
    # clock gate during the initial DMA wait.
    s8_0 = stp8.tile([P, KT, L], FP8, tag="s8")
    t8_0 = stp8.tile([P, KT, N], FP8, tag="t8")
    nc.sync.dma_start(
        w12[:, :, 0:P], w12_d.ap()[0].rearrange("(kk p) c -> p kk c", p=P))
    for k in range(KT):
        nc.sync.dma_start(
            s8_0[:, k, 0:LC], src8_d.ap()[0, k * P:(k + 1) * P, 0:LC])
    warm_ps = psum.tile([P, LC], F32, name="ps0", tag="ps0")
    for _ in range(10):
        nc.tensor.matmul(warm_ps[:, :256], onesbig[:, :, :P],
                         onesbig[:, :, :256],
                         start=True, stop=True, perf_mode=DR)
    g8_0 = actp.tile([P, KT, L], FP8, tag="g8")
    for j in range(KT):
        if j + 1 < KT:
            nc.sync.dma_start(
                w12[:, :, (j + 1) * P:(j + 2) * P],
                w12_d.ap()[j + 1].rearrange("(kk p) c -> p kk c", p=P))
        ps = psum.tile([P, LC], F32, name="ps0", tag="ps0")
        for kp in range(KP):
            nc.tensor.matmul(
                ps[:], w12[:, 2 * kp:2 * kp + 2, j * P:(j + 1) * P],
                s8_0[:, 2 * kp:2 * kp + 2, 0:LC],
                start=(kp == 0), stop=(kp == KP - 1), perf_mode=DR)
        nc.scalar.activation(g8_0[:, j, 0:LC], ps[:], AF.Identity,
                             scale=G_DRAIN)
    for k in range(KT):
        nc.sync.dma_start(
            s8_0[:, k, LC:L], src8_d.ap()[0, k * P:(k + 1) * P, LC:L])
    phase_a(s8_0, g8_0, [1])
    _load_act(nc, t8_0, trgT_d, 0)
    tn_0 = stp1.tile([P, KT, N], FP8, tag="tn", bufs=2)
    _load_act(nc, tn_0, trgN_d, 0)
    _load_w(nc, wfuse, wfuse_d, KT)
    sb_0 = stp1.tile([P, KT, L], BF16, tag="sb")
    _load_act(nc, sb_0, srcb_d, 0)
    _load_w(nc, wcorr, wcorr_d, KT)
    _load_w(nc, w3b, w3b_d, KT)

    nxt = dict(s8=s8_0, t8=t8_0, sb=sb_0, tn=tn_0)
    g8_next = g8_0   # A(b) is emitted during F(b-1); A(0) in the prologue
    for b in range(BS):
        s8, t8, sb, tn = nxt["s8"], nxt["t8"], nxt["sb"], nxt["tn"]
        g8 = g8_next

        # ---- B: score_T[n, l]; e = exp(score/32 + beta) -> fp8 ----
        e8 = actp.tile([P, KT, L], FP8, tag="e8")
        for i in range(KT):
            pss = [psum.tile([P, LC], F32, name=f"ps{lcx}", tag=f"ps{lcx}")
                   for lcx in range(N_LC)]
            for kp in range(KP):
                for lc in range(N_LC):
                    nc.tensor.matmul(
                        pss[lc][:], t8[:, 2 * kp:2 * kp + 2, i * P:(i + 1) * P],
                        g8[:, 2 * kp:2 * kp + 2, lc * LC:(lc + 1) * LC],
                        start=(kp == 0), stop=(kp == KP - 1), perf_mode=DR)
            for lc in range(N_LC):
                nc.scalar.activation(
                    e8[:, i, lc * LC:(lc + 1) * LC], pss[lc][:], AF.Exp,
                    scale=E_DRAIN, bias=betafull[:, b * KT + i:b * KT + i + 1])

        # ---- C: ctxd_T[d, l] = trgN8.T @ e8, drain x 0.5 -> fp8 ----
        cx8 = actp.tile([P, KT, L], FP8, tag="cx8")
        for j in range(KT):
            pss = [psum.tile([P, LC], F32, name=f"ps{lcx}", tag=f"ps{lcx}")
                   for lcx in range(N_LC)]
            for ip in range(KP):
                for lc in range(N_LC):
                    nc.tensor.matmul(
                        pss[lc][:], tn[:, 2 * ip:2 * ip + 2, j * P:(j + 1) * P],
                        e8[:, 2 * ip:2 * ip + 2, lc * LC:(lc + 1) * LC],
                        start=(ip == 0), stop=(ip == KP - 1), perf_mode=DR)
            for lc in range(N_LC):
                nc.scalar.activation(
                    cx8[:, j, lc * LC:(lc + 1) * LC], pss[lc][:], AF.Identity,
                    scale=CS)

        if b + 1 < BS:
            nxt = dict(
                s8=stp8.tile([P, KT, L], FP8, name="s8n", tag="s8"),
                t8=stp8.tile([P, KT, N], FP8, name="t8n", tag="t8"),
                tn=stp1.tile([P, KT, N], FP8, name="tnn", tag="tn", bufs=2),
                sb=stp1.tile([P, KT, L], BF16, name="sbn", tag="sb"))
            _load_act(nc, nxt["s8"], src8_d, b + 1)
            _load_act(nc, nxt["t8"], trgT_d, b + 1)
            _load_act(nc, nxt["tn"], trgN_d, b + 1)
            _load_act(nc, nxt["sb"], srcb_d, b + 1)

        # ---- E: pre_T[o, l] = Wfuse.T @ srcTb + bh2 (bf16, independent) ----
        pre = actp.tile([P, KT, L], BF16, tag="pre")
        for j in range(KT):
            pss = [psum.tile([P, LC], F32, name=f"ps{lcx}", tag=f"ps{lcx}")
                   for lcx in range(N_LC)]
            for k in range(KT):
                for lc in range(N_LC):
                    nc.tensor.matmul(
                        pss[lc][:], wfuse[:, k, j * P:(j + 1) * P],
                        sb[:, k, lc * LC:(lc + 1) * LC],
                        start=(k == 0), stop=(k == KT - 1))
            for lc in range(N_LC):
                nc.scalar.activation(
                    pre[:, j, lc * LC:(lc + 1) * LC], pss[lc][:], AF.Identity,
                    bias=bh2col[:, j:j + 1])

        # denominator: DR ones-matrix partition-reduce (2 N-tiles per MM);
        # every d_ps row holds 128*denom, so the reciprocal lands already
        # broadcast: rbc = 2^-7/denom (2^-7 folds the Wcorr/ctxd scales).
        rbcs = []
        for lc in range(N_LC):
            d_ps = auxps.tile([P, LC], F32, tag="dps")
            for ip in range(KP):
                nc.tensor.matmul(
                    d_ps[:], onesbig[:, :, :P],
                    e8[:, 2 * ip:2 * ip + 2, lc * LC:(lc + 1) * LC],
                    start=(ip == 0), stop=(ip == KP - 1), perf_mode=DR)
            rbc = smallp.tile([P, LC], F32, tag="rbc")
            nc.vector.reciprocal(rbc[:], d_ps[:])
            rbcs.append(rbc)

        # ---- D: corr; h2 = relu(pre - corr/denom) -> bf16 ----
        h2 = actp.tile([P, KT, L], BF16, tag="h2")
        for j in range(KT):
            pss = [psum.tile([P, LC], F32, name=f"ps{lcx}", tag=f"ps{lcx}")
                   for lcx in range(N_LC)]
            for kp in range(KP):
                for lc in range(N_LC):
                    nc.tensor.matmul(
                        pss[lc][:], wcorr[:, 2 * kp:2 * kp + 2, j * P:(j + 1) * P],
                        cx8[:, 2 * kp:2 * kp + 2, lc * LC:(lc + 1) * LC],
                        start=(kp == 0), stop=(kp == KP - 1), perf_mode=DR)
            for lc in range(N_LC):
                lsl = slice(lc * LC, (lc + 1) * LC)
                tmp = smallp.tile([P, LC], F32, tag="tmp")
                nc.vector.tensor_mul(tmp[:], pss[lc][:], rbcs[lc][:])
                hsum = smallp.tile([P, LC], F32, tag="hsum")
                nc.vector.tensor_sub(hsum[:], pre[:, j, lsl], tmp[:])
                nc.scalar.activation(h2[:, j, lsl], hsum[:], AF.Relu)

        # ---- F: out[l, o] = h2.T @ W3b + b3b; A(b+1) j-groups are
        # interleaved so either phase's drain bubbles fill with the other's
        # matmuls (software pipeline across batches).
        if b + 1 < BS:
            g8_next = actp.tile([P, KT, L], FP8, name="g8n", tag="g8")
        fidx = 0
        for lc in range(N_LC):
            for lt in range(LC // P):
                lab = lc * LC + lt * P
                pss = [psum.tile([P, LC], F32, name=f"ps{lcx}", tag=f"ps{lcx}")
                       for lcx in range(N_LC)]
                for k in range(KT):
                    for oc in range(O // LC):
                        nc.tensor.matmul(
                            pss[oc][:], h2[:, k, lab:lab + P],
                            w3b[:, k, oc * LC:(oc + 1) * LC],
                            start=(k == 0), stop=(k == KT - 1))
                for oc in range(O // LC):
                    o_sb = outp.tile([P, LC], F32, tag="osb")
                    nc.vector.tensor_add(o_sb[:], pss[oc][:],
                                         b3bfull[:, oc * LC:(oc + 1) * LC])
                    nc.sync.dma_start(
                        out.ap()[b, lab:lab + P, oc * LC:(oc + 1) * LC], o_sb[:])
                if b + 1 < BS:
                    jn = fidx
                    fidx += 1
                    psn = {lcx: psum.tile([P, LC], F32, name=f"ps{lcx}",
                                          tag=f"ps{lcx}")
                           for lcx in range(N_LC)}
                    for kp in range(KP):
                        for lcn in range(N_LC):
                            nc.tensor.matmul(
                                psn[lcn][:],
                                w12[:, 2 * kp:2 * kp + 2, jn * P:(jn + 1) * P],
                                nxt["s8"][:, 2 * kp:2 * kp + 2,
                                          lcn * LC:(lcn + 1) * LC],
                                start=(kp == 0), stop=(kp == KP - 1),
                                perf_mode=DR)
                    for lcn in range(N_LC):
                        nc.scalar.activation(
                            g8_next[:, jn, lcn * LC:(lcn + 1) * LC],
                            psn[lcn][:], AF.Identity, scale=G_DRAIN)

    ctx.close()


_NC_CACHE = None


def _get_nc():
    global _NC_CACHE
    if _NC_CACHE is None:
        nc = bacc.Bacc("TRN2", target_bir_lowering=False, debug=False,
                       num_devices=N_CORES)
        with tile.TileContext(nc) as tc:
            _build(nc, tc)
        nc.compile()
        _NC_CACHE = nc
    return _NC_CACHE


def _q8(x, scale=1.0):
    y = np.asarray(x, np.float32) * np.float32(scale)
    np.clip(y, -240.0, 240.0, out=y)
    return y.astype(NP_FP8)


def kernel(**inputs):
    nc = _get_nc()
    src = np.asarray(inputs["src"], dtype=np.float32)
    trg = np.asarray(inputs["trg"], dtype=np.float32)
    W1 = np.asarray(inputs["W1"], np.float32)
    b1 = np.asarray(inputs["b1"], np.float32)
    W2 = np.asarray(inputs["W2"], np.float32)
    b2 = np.asarray(inputs["b2"], np.float32)
    W3a = np.asarray(inputs["W3a"], np.float32)
    b3a = np.asarray(inputs["b3a"], np.float32)
    W3b = np.asarray(inputs["W3b"], np.float32)
    b3b = np.asarray(inputs["b3b"], np.float32)

    W3aT, W3aB = W3a[:O], W3a[O:]
    W12 = W1 @ W2.T
    Wfuse = W1 @ (W3aT + W3aB)
    Wcorr = W1 @ W3aB
    bh2 = b1 @ W3aT + b3a
    beta = (trg @ (W2 @ b1) + np.dot(b1, b2)).astype(np.float32)  # (B, N)

    src_t = np.ascontiguousarray(src.transpose(0, 2, 1))   # (B, D, L)
    trg_t = np.ascontiguousarray(trg.transpose(0, 2, 1))   # (B, D, N)
    shared = {
        "W12s": np.ascontiguousarray(
            _q8(W12, WS).reshape(D, KT, P).transpose(1, 0, 2)),
        "Wfuse": np.ascontiguousarray(Wfuse.astype(NP_BF16)),
        "Wcorrs": np.ascontiguousarray(_q8(Wcorr, WCS)),
        "W3bb": np.ascontiguousarray(W3b.astype(NP_BF16)),
        "bh2": np.ascontiguousarray(bh2),
        "b3bf": np.ascontiguousarray(b3b),
    }
    src_t8 = _q8(src_t)
    src_tb = src_t.astype(NP_BF16)
    trg_t8 = _q8(trg_t)
    trg_n8 = _q8(trg)
    in_maps = []
    for c in range(N_CORES):
        m = dict(shared)
        s = slice(c * BS, (c + 1) * BS)
        m["srcT8"] = src_t8[s]
        m["srcTb"] = src_tb[s]
        m["trgT8"] = trg_t8[s]
        m["trgN8"] = trg_n8[s]
        m["beta"] = np.ascontiguousarray(beta[s])
        in_maps.append(m)
    res = run_bass_kernel_spmd(nc, in_maps, core_ids=list(range(N_CORES)))
    return np.concatenate([r["out"] for r in res.results], axis=0)


# revision 15
# speedup vs baseline: 1.0068x; 1.0025x over previous
"""Trainium2 Bass kernel for nn_DiffModule_40827959116531 (sparse_attention).

Reference (per batch element):
    sv  = src @ W1 + b1;  tk = trg @ W2 + b2;  tv = trg @ W1 + b1
    score = sv @ tk.T / sqrt(O);  prob = softmax(score)
    ctx = prob @ tv;  h = [sv, sv - ctx]
    out = relu(h @ W3a + b3a) @ W3b + b3b

Algebraic restructuring (host-precomputed fused weights; exact up to the
shift-invariance of softmax which absorbs the b2 term):
    W12   = W1 @ W2.T               score = (src @ W12) @ trg.T + beta
    beta  = trg @ (W2 @ b1) + b1.b2         (per-target logit bias)
    Wfuse = W1 @ (W3aTop + W3aBot)  h @ W3a = src@Wfuse - ctx@W3aBot + bias
    Wcorr = W1 @ W3aBot             ctx@W3aBot = ((e@trg)/denom) @ Wcorr + ..
    bh2   = b1 @ W3aTop + b3a
This cuts the 8 matmul-units/batch to 6, and the 4 units feeding only the
softmax/correction path (g, score, ctxd, corr) tolerate fp8 -> run them as
fp8e4 DoubleRow (2 K-chunks per instruction). Only pre=src@Wfuse and the
final h2@W3b stay bf16. Verified vs reference in numpy: rel ~4e-3.

Scaling (fp8e4 min-normal 2^-6, TRN max +-240): W12 pre-scaled 4096, Wcorr
256; g stored x8 (drain scale 2^-9); exp drain scale 2^-8; ctxd stored
x0.5. The denominator ones-matrix holds 128.0 so reciprocal(d_ps) is
exactly the corr multiplier 1/(128*denom) broadcast on all partitions.

Sharding: data-parallel over B=32 across 8 cores (4 batch elems each).
"""

import math
from contextlib import ExitStack

import ml_dtypes
import numpy as np

import concourse.bass as bass
import concourse.mybir as mybir
import concourse.tile as tile
from concourse import bacc
from concourse.bass_utils import run_bass_kernel_spmd

P = 128
B_FULL = 32
N_CORES = 8
BS = B_FULL // N_CORES  # 4 batch elements per core
L = 1024
N = 1024
D = 1024
O = 1024

F32 = mybir.dt.float32
BF16 = mybir.dt.bfloat16
FP8 = mybir.dt.float8e4
AF = mybir.ActivationFunctionType
DR = mybir.MatmulPerfMode.DoubleRow
NP_BF16 = ml_dtypes.bfloat16
NP_FP8 = ml_dtypes.float8_e4m3fn

LC = 512
N_LC = L // LC            # 2 moving chunks of 512
KT = 8                    # 128-tiles along any contraction dim
KP = KT // 2              # DoubleRow pairs

WS = 4096.0               # host pre-scale on W12
WCS = 256.0               # host pre-scale on Wcorr
GS = 8.0                  # g storage scale
CS = 0.5                  # ctxd storage scale
G_DRAIN = GS / WS                     # 2^-9
E_DRAIN = 1.0 / (GS * math.sqrt(O))   # 2^-8


def _load_w(nc, dst, w_dram, ktiles):
    for k in range(ktiles):
        nc.sync.dma_start(dst[:, k, :], w_dram.ap()[k * P:(k + 1) * P, :])


def _load_act(nc, dest, dram, b):
    for k in range(KT):
        nc.sync.dma_start(dest[:, k, :], dram.ap()[b, k * P:(k + 1) * P, :])


def _build(nc, tc):
    src8_d = nc.dram_tensor("srcT8", [BS, D, L], FP8, kind="ExternalInput")
    srcb_d = nc.dram_tensor("srcTb", [BS, D, L], BF16, kind="ExternalInput")
    trgT_d = nc.dram_tensor("trgT8", [BS, D, N], FP8, kind="ExternalInput")
    trgN_d = nc.dram_tensor("trgN8", [BS, N, D], FP8, kind="ExternalInput")
    w12_d = nc.dram_tensor("W12s", [KT, D, P], FP8, kind="ExternalInput")
    wfuse_d = nc.dram_tensor("Wfuse", [D, O], BF16, kind="ExternalInput")
    wcorr_d = nc.dram_tensor("Wcorrs", [D, O], FP8, kind="ExternalInput")
    w3b_d = nc.dram_tensor("W3bb", [O, O], BF16, kind="ExternalInput")
    bh2_d = nc.dram_tensor("bh2", [O], F32, kind="ExternalInput")
    b3b_d = nc.dram_tensor("b3bf", [O], F32, kind="ExternalInput")
    beta_d = nc.dram_tensor("beta", [BS, N], F32, kind="ExternalInput")
    out = nc.dram_tensor("out", [BS, L, O], BF16, kind="ExternalOutput")

    ctx = ExitStack()
    singles = ctx.enter_context(tc.tile_pool(name="singles", bufs=1))
    stp8 = ctx.enter_context(tc.tile_pool(name="stp8", bufs=2))
    stp1 = ctx.enter_context(tc.tile_pool(name="stp1", bufs=1))
    actp = ctx.enter_context(tc.tile_pool(name="actp", bufs=1))
    smallp = ctx.enter_context(tc.tile_pool(name="smallp", bufs=2))
    outp = ctx.enter_context(tc.tile_pool(name="outp", bufs=6))
    psum = ctx.enter_context(tc.tile_pool(name="psum", bufs=3, space="PSUM"))
    auxps = ctx.enter_context(tc.tile_pool(name="auxps", bufs=2, space="PSUM"))

    # ---- constants ----
    w12 = singles.tile([P, KT, D], FP8)
    wfuse = singles.tile([P, KT, O], BF16)
    wcorr = singles.tile([P, KT, O], FP8)
    w3b = singles.tile([P, KT, O], BF16)
    bh2col = singles.tile([P, KT], F32)
    b3bfull = singles.tile([P, O], F32)
    betafull = singles.tile([P, BS * KT], F32)
    onesbig = singles.tile([P, 2, N], FP8)

    nc.sync.dma_start(bh2col[:], bh2_d.ap().rearrange("(oo op) -> op oo", op=P))
    nc.sync.dma_start(
        betafull[:], beta_d.ap().rearrange("b (no np) -> np (b no)", np=P))
    nc.sync.dma_start(
        b3bfull[:], bass.AP(tensor=b3b_d.ap().tensor, offset=0, ap=[[0, P], [1, O]]))
    nc.vector.memset(onesbig[:], 128.0)

    def phase_a(s8, g8, lcs):
        """g_T[d2, l] = W12s.T @ srcT8, drain x 2^-9 -> fp8 (x GS)."""
        for j in range(KT):
            pss = {lcx: psum.tile([P, LC], F32, name=f"ps{lcx}", tag=f"ps{lcx}")
                   for lcx in lcs}
            for kp in range(KP):
                for lc in lcs:
                    nc.tensor.matmul(
                        pss[lc][:], w12[:, 2 * kp:2 * kp + 2, j * P:(j + 1) * P],
                        s8[:, 2 * kp:2 * kp + 2, lc * LC:(lc + 1) * LC],
                        start=(kp == 0), stop=(kp == KP - 1), perf_mode=DR)
            for lc in lcs:
                nc.scalar.activation(
                    g8[:, j, lc * LC:(lc + 1) * LC], pss[lc][:], AF.Identity,
                    scale=G_DRAIN)

    # batch-0: W12 block 0 + the first half of srcT8 go first; subsequent
    # W12 column-blocks are emitted between A(j) groups so A(j) only waits
    # for blocks <= j. Junk matmuls on the resident ones tile warm the HAM
    # clock gate during the initial DMA wait.
    s8_0 = stp8.tile([P, KT, L], FP8, tag="s8")
    t8_0 = stp8.tile([P, KT, N], FP8, tag="t8")
    nc.sync.dma_start(
        w12[:, :, 0:P], w12_d.ap()[0].rearrange("(kk p) c -> p kk c", p=P))
    for k in range(KT):
        nc.sync.dma_start(
            s8_0[:, k, 0:LC], src8_d.ap()[0, k * P:(k + 1) * P, 0:LC])
    warm_ps = psum.tile([P, LC], F32, name="ps0", tag="ps0")
    for _ in range(10):
        nc.tensor.matmul(warm_ps[:, :256], onesbig[:, :, :P],
                         onesbig[:, :, :256],
                         start=True, stop=True, perf_mode=DR)
    g8_0 = actp.tile([P, KT, L], FP8, tag="g8")
    for j in range(KT):
        if j + 1 < KT:
            nc.sync.dma_start(
                w12[:, :, (j + 1) * P:(j + 2) * P],
                w12_d.ap()[j + 1].rearrange("(kk p) c -> p kk c", p=P))
        ps = psum.tile([P, LC], F32, name="ps0", tag="ps0")
        for kp in range(KP):
            nc.tensor.matmul(
                ps[:], w12[:, 2 * kp:2 * kp + 2, j * P:(j + 1) * P],
                s8_0[:, 2 * kp:2 * kp + 2, 0:LC],
                start=(kp == 0), stop=(kp == KP - 1), perf_mode=DR)
        nc.scalar.activation(g8_0[:, j, 0:LC], ps[:], AF.Identity,
                             scale=G_DRAIN)
    for k in range(KT):
        nc.sync.dma_start(
            s8_0[:, k, LC:L], src8_d.ap()[0, k * P:(k + 1) * P, LC:L])
    phase_a(s8_0, g8_0, [1])
    _load_act(nc, t8_0, trgT_d, 0)
    tn_0 = stp1.tile([P, KT, N], FP8, tag="tn", bufs=2)
    _load_act(nc, tn_0, trgN_d, 0)
    _load_w(nc, wfuse, wfuse_d, KT)
    sb_0 = stp1.tile([P, KT, L], BF16, tag="sb")
    _load_act(nc, sb_0, srcb_d, 0)
    _load_w(nc, wcorr, wcorr_d, KT)
    _load_w(nc, w3b, w3b_d, KT)

    nxt = dict(s8=s8_0, t8=t8_0, sb=sb_0, tn=tn_0)
    g8_next = g8_0   # A(b) is emitted during F(b-1); A(0) in the prologue
    for b in range(BS):
        s8, t8, sb, tn = nxt["s8"], nxt["t8"], nxt["sb"], nxt["tn"]
        g8 = g8_next

        # ---- B: score_T[n, l]; e = exp(score/32 + beta) -> fp8 ----
        e8 = actp.tile([P, KT, L], FP8, tag="e8")
        for i in range(KT):
            pss = [psum.tile([P, LC], F32, name=f"ps{lcx}", tag=f"ps{lcx}")
                   for lcx in range(N_LC)]
            for kp in range(KP):
                for lc in range(N_LC):
                    nc.tensor.matmul(
                        pss[lc][:], t8[:, 2 * kp:2 * kp + 2, i * P:(i + 1) * P],
                        g8[:, 2 * kp:2 * kp + 2, lc * LC:(lc + 1) * LC],
                        start=(kp == 0), stop=(kp == KP - 1), perf_mode=DR)
            for lc in range(N_LC):
                nc.scalar.activation(
                    e8[:, i, lc * LC:(lc + 1) * LC], pss[lc][:], AF.Exp,
                    scale=E_DRAIN, bias=betafull[:, b * KT + i:b * KT + i + 1])

        # ---- C: ctxd_T[d, l] = trgN8.T @ e8, drain x 0.5 -> fp8 ----
        cx8 = actp.tile([P, KT, L], FP8, tag="cx8")
        for j in range(KT):
            pss = [psum.tile([P, LC], F32, name=f"ps{lcx}", tag=f"ps{lcx}")
                   for lcx in range(N_LC)]
            for ip in range(KP):
                for lc in range(N_LC):
                    nc.tensor.matmul(
                        pss[lc][:], tn[:, 2 * ip:2 * ip + 2, j * P:(j + 1) * P],
                        e8[:, 2 * ip:2 * ip + 2, lc * LC:(lc + 1) * LC],
                        start=(ip == 0), stop=(ip == KP - 1), perf_mode=DR)
            for lc in range(N_LC):
                nc.scalar.activation(
                    cx8[:, j, lc * LC:(lc + 1) * LC], pss[lc][:], AF.Identity,
                    scale=CS)

        if b + 1 < BS:
            nxt = dict(
                s8=stp8.tile([P, KT, L], FP8, name="s8n", tag="s8"),
                t8=stp8.tile([P, KT, N], FP8, name="t8n", tag="t8"),
                tn=stp1.tile([P, KT, N], FP8, name="tnn", tag="tn", bufs=2),
                sb=stp1.tile([P, KT, L], BF16, name="sbn", tag="sb"))
            _load_act(nc, nxt["s8"], src8_d, b + 1)
            _load_act(nc, nxt["t8"], trgT_d, b + 1)
            _load_act(nc, nxt["tn"], trgN_d, b + 1)
            _load_act(nc, nxt["sb"], srcb_d, b + 1)

        # ---- E: pre_T[o, l] = Wfuse.T @ srcTb + bh2 (bf16, independent) ----
        pre = actp.tile([P, KT, L], BF16, tag="pre")
        for j in range(KT):
            pss = [psum.tile([P, LC], F32, name=f"ps{lcx}", tag=f"ps{lcx}")
                   for lcx in range(N_LC)]
            for k in range(KT):
                for lc in range(N_LC):
                    nc.tensor.matmul(
                        pss[lc][:], wfuse[:, k, j * P:(j + 1) * P],
                        sb[:, k, lc * LC:(lc + 1) * LC],
                        start=(k == 0), stop=(k == KT - 1))
            for lc in range(N_LC):
                nc.scalar.activation(
                    pre[:, j, lc * LC:(lc + 1) * LC], pss[lc][:], AF.Identity,
                    bias=bh2col[:, j:j + 1])

        # denominator: DR ones-matrix partition-reduce (2 N-tiles per MM);
        # every d_ps row holds 128*denom, so the reciprocal lands already
        # broadcast: rbc = 2^-7/denom (2^-7 folds the Wcorr/ctxd scales).
        rbcs = []
        for lc in range(N_LC):
            d_ps = auxps.tile([P, LC], F32, tag="dps")
            for ip in range(KP):
                nc.tensor.matmul(
                    d_ps[:], onesbig[:, :, :P],
                    e8[:, 2 * ip:2 * ip + 2, lc * LC:(lc + 1) * LC],
                    start=(ip == 0), stop=(ip == KP - 1), perf_mode=DR)
            rbc = smallp.tile([P, LC], F32, tag="rbc")
            nc.vector.reciprocal(rbc[:], d_ps[:])
            rbcs.append(rbc)

        # ---- D: corr; h2 = relu(pre - corr/denom) -> bf16 ----
        h2 = actp.tile([P, KT, L], BF16, tag="h2")
        for j in range(KT):
            pss = [psum.tile([P, LC], F32, name=f"ps{lcx}", tag=f"ps{lcx}")
                   for lcx in range(N_LC)]
            for kp in range(KP):
                for lc in range(N_LC):
                    nc.tensor.matmul(
                        pss[lc][:], wcorr[:, 2 * kp:2 * kp + 2, j * P:(j + 1) * P],
                        cx8[:, 2 * kp:2 * kp + 2, lc * LC:(lc + 1) * LC],
                        start=(kp == 0), stop=(kp == KP - 1), perf_mode=DR)
            for lc in range(N_LC):
                lsl = slice(lc * LC, (lc + 1) * LC)
                tmp = smallp.tile([P, LC], F32, tag="tmp")
                nc.vector.tensor_mul(tmp[:], pss[lc][:], rbcs[lc][:])
                hsum = smallp.tile([P, LC], F32, tag="hsum")
                nc.vector.tensor_sub(hsum[:], pre[:, j, lsl], tmp[:])
                nc.scalar.activation(h2[:, j, lsl], hsum[:], AF.Relu)

        # ---- F: out[l, o] = h2.T @ W3b + b3b; A(b+1) j-groups are
        # interleaved so either phase's drain bubbles fill with the other's
        # matmuls (software pipeline across batches).
        if b + 1 < BS:
            g8_next = actp.tile([P, KT, L], FP8, name="g8n", tag="g8")
        fidx = 0
        for lc in range(N_LC):
            for lt in range(LC // P):
                lab = lc * LC + lt * P
                pss = [psum.tile([P, LC], F32, name=f"ps{lcx}", tag=f"ps{lcx}")
                       for lcx in range(N_LC)]
                for k in range(KT):
                    for oc in range(O // LC):
                        nc.tensor.matmul(
                            pss[oc][:], h2[:, k, lab:lab + P],
                            w3b[:, k, oc * LC:(oc + 1) * LC],
                            start=(k == 0), stop=(k == KT - 1))
                for oc in range(O // LC):
                    o_sb = outp.tile([P, LC], BF16, tag="osb")
                    nc.vector.tensor_add(o_sb[:], pss[oc][:],
                                         b3bfull[:, oc * LC:(oc + 1) * LC])
                    nc.sync.dma_start(
                        out.ap()[b, lab:lab + P, oc * LC:(oc + 1) * LC], o_sb[:])
                if b + 1 < BS:
                    jn = fidx
                    fidx += 1
                    psn = {lcx: psum.tile([P, LC], F32, name=f"ps{lcx}",
                                          tag=f"ps{lcx}")
                           for lcx in range(N_LC)}
                    for kp in range(KP):
                        for lcn in range(N_LC):
                            nc.tensor.matmul(
                                psn[lcn][:],
                                w12[:, 2 * kp:2 * kp + 2, jn * P:(jn + 1) * P],
                                nxt["s8"][:, 2 * kp:2 * kp + 2,
                                          lcn * LC:(lcn + 1) * LC],
                                start=(kp == 0), stop=(kp == KP - 1),
                                perf_mode=DR)
                    for lcn in range(N_LC):
                        nc.scalar.activation(
                            g8_next[:, jn, lcn * LC:(lcn + 1) * LC],
                            psn[lcn][:], AF.Identity, scale=G_DRAIN)

    ctx.close()


_NC_CACHE = None


def _get_nc():
    global _NC_CACHE
    if _NC_CACHE is None:
        nc = bacc.Bacc("TRN2", target_bir_lowering=False, debug=False,
                       num_devices=N_CORES)
        with tile.TileContext(nc) as tc:
            _build(nc, tc)
        nc.compile()
        _NC_CACHE = nc
    return _NC_CACHE


def _q8(x, scale=1.0):
    y = np.asarray(x, np.float32) * np.float32(scale)
    np.clip(y, -240.0, 240.0, out=y)
    return y.astype(NP_FP8)


def kernel(**inputs):
    nc = _get_nc()
    src = np.asarray(inputs["src"], dtype=np.float32)
    trg = np.asarray(inputs["trg"], dtype=np.float32)
    W1 = np.asarray(inputs["W1"], np.float32)
    b1 = np.asarray(inputs["b1"], np.float32)
    W2 = np.asarray(inputs["W2"], np.float32)
    b2 = np.asarray(inputs["b2"], np.float32)
    W3a = np.asarray(inputs["W3a"], np.float32)
    b3a = np.asarray(inputs["b3a"], np.float32)
    W3b = np.asarray(inputs["W3b"], np.float32)
    b3b = np.asarray(inputs["b3b"], np.float32)

    W3aT, W3aB = W3a[:O], W3a[O:]
    W12 = W1 @ W2.T
    Wfuse = W1 @ (W3aT + W3aB)
    Wcorr = W1 @ W3aB
    bh2 = b1 @ W3aT + b3a
    beta = (trg @ (W2 @ b1) + np.dot(b1, b2)).astype(np.float32)  # (B, N)

    src_t = np.ascontiguousarray(src.transpose(0, 2, 1))   # (B, D, L)
    trg_t = np.ascontiguousarray(trg.transpose(0, 2, 1))   # (B, D, N)
    shared = {
        "W12s": np.ascontiguousarray(
            _q8(W12, WS).reshape(D, KT, P).transpose(1, 0, 2)),
        "Wfuse": np.ascontiguousarray(Wfuse.astype(NP_BF16)),
        "Wcorrs": np.ascontiguousarray(_q8(Wcorr, WCS)),
        "W3bb": np.ascontiguousarray(W3b.astype(NP_BF16)),
        "bh2": np.ascontiguousarray(bh2),
        "b3bf": np.ascontiguousarray(b3b),
    }
    src_t8 = _q8(src_t)
    src_tb = src_t.astype(NP_BF16)
    trg_t8 = _q8(trg_t)
    trg_n8 = _q8(trg)
    in_maps = []
    for c in range(N_CORES):
        m = dict(shared)
        s = slice(c * BS, (c + 1) * BS)
        m["srcT8"] = src_t8[s]
        m["srcTb"] = src_tb[s]
        m["trgT8"] = trg_t8[s]
        m["trgN8"] = trg_n8[s]
        m["beta"] = np.ascontiguousarray(beta[s])
        in_maps.append(m)
    res = run_bass_kernel_spmd(nc, in_maps, core_ids=list(range(N_CORES)))
    return np.concatenate([r["out"] for r in res.results],
                          axis=0).astype(np.float32)
